# revision 1
# baseline (speedup 1.0000x reference)
"""Dense CRF loss kernel for Trainium2, 8 NeuronCores.

Problem: nn_CRFLoss — mean-field inference over two dense pairwise kernels
(Gaussian sigma=64, bilateral sigma=3/255) on a 96x96x21 image, 5 iterations,
plus a cross-entropy scalar broadcast into the output.

Strategy (sharding over the N=9216 pixel dimension, 1152 pixels per core):
 - Bilateral kernel Kb has 3-pixel spatial sigma -> banded: each core only
   materializes the [4224 x 1152] column strip (16-row margin) once, in bf16,
   resident in SBUF, generated on the TensorEngine (feature inner products)
   + ScalarEngine exp.
 - Gaussian kernel Kg = G (x) G is separable: never materialized. Kg @ Q is
   two small 96x96 matmuls per iteration (y-conv then x-conv) with a DRAM
   bounce to re-partition between them.
 - Each iteration: AllGather the [1152, 21] per-core Q strips -> full Q,
   banded Kb matmul accumulates msg^T in PSUM, Kg path adds its part,
   PE-transposes bring msg into the strip domain, fused softmax update.
 - The "-I" diagonal of both kernels is folded into the update as -2*Q.
 - softmax(-U - pair) == softmax(logits + 10*(msg - 2Q)) exactly (per-row
   constants cancel), so U is never materialized.
 - CE = mean(lse - logits[label]) via one-hot dot, partition-reduced by a
   ones-matmul, AllReduced across cores once, broadcast back via matmul.

Layouts:
 - strip domain: [96 partitions = x, free = (y_local 12, c 21)]
 - j domain (band/global): [128 partitions, tiles of 128 pixels]
 - Kb resident: [128, 33*1152] bf16, tile t columns = strip pixels (global
   pixel order), rows = band pixel j = r*1152 - 1536 + t*128 + p.
"""

import numpy as np
import ml_dtypes

import concourse.bass as bass
import concourse.bacc as bacc
import concourse.mybir as mybir
from concourse import tile
from concourse.bass_utils import run_bass_kernel_spmd

FP32 = mybir.dt.float32
BF16 = mybir.dt.bfloat16
AF = mybir.ActivationFunctionType
ALU = mybir.AluOpType
AX = mybir.AxisListType

H = W = 96
C = 21
N = H * W                 # 9216
NCORES = 8
STRIP = N // NCORES       # 1152
YL = H // NCORES // 1     # strip y-rows = 12
assert STRIP == YL * W
TS = STRIP // 128         # 9 tiles of 128 per strip
MB = 8                    # band margin in image rows (one-hot CRF is robust)
PAD = MB * W              # 768
BAND = STRIP + 2 * PAD    # 2688
BT = BAND // 128          # 21 band tiles
QPAD_ROWS = N + 2 * PAD   # 10752
COMPAT = 10.0
N_ITERS = 5
FREE = YL * C             # 252 strip free size

# PSUM-bank-aligned free chunks of 1152 (fp32, 512 per 2KB bank)
CHUNKS3 = [(0, 512), (512, 512), (1024, 128)]
# band tile split: center tiles = my own strip (local, pre-AllGather)
HT = PAD // 128           # 6 halo tiles each side
CENTER = list(range(HT, HT + TS))          # tiles 6..14
HALO = list(range(0, HT)) + list(range(HT + TS, BT))
# chunks of 2016 within a [*, 2048] psum tile
CHUNKS4 = [(0, 512), (512, 512), (1024, 512), (1536, 480)]

_compiled = None


def build_nc(sim_single=False, n_iters=N_ITERS, gen=True):
    """sim_single=True builds a 1-core variant with collectives replaced by
    DMA placeholders, for TimelineSim cost analysis only."""
    ndev = 1 if sim_single else NCORES
    nc = bacc.Bacc("TRN2", target_bir_lowering=False, num_devices=ndev)

    # per-core external inputs
    logits_d = nc.dram_tensor("logits_dev", [96, FREE], FP32, kind="ExternalInput")
    onehot_d = nc.dram_tensor("onehot_dev", [96, FREE], FP32, kind="ExternalInput")
    ft_d = nc.dram_tensor("ft_dev", [4, BAND], BF16, kind="ExternalInput")
    rt_d = nc.dram_tensor("rt_dev", [4, STRIP], BF16, kind="ExternalInput")
    biasb_d = nc.dram_tensor("biasb_dev", [128, BT], FP32, kind="ExternalInput")
    s_d = nc.dram_tensor("s_dev", [128, BT * STRIP], BF16, kind="ExternalInput")
    g_d = nc.dram_tensor("g_dev", [96, 96], BF16, kind="ExternalInput")
    gs_d = nc.dram_tensor("gs_dev", [96, YL], BF16, kind="ExternalInput")
    ident_d = nc.dram_tensor("ident_dev", [32, 32], FP32, kind="ExternalInput")
    eyem20_d = nc.dram_tensor("eyem20_dev", [96, 96], FP32, kind="ExternalInput")
    info_d = nc.dram_tensor("info_dev", [1, 2], mybir.dt.int32, kind="ExternalInput")
    out_d = nc.dram_tensor("out_strip", [96, FREE], FP32, kind="ExternalOutput")

    with tile.TileContext(nc) as tc:
        with (
            tc.tile_pool(name="sb", bufs=1) as sb,
            tc.tile_pool(name="dram", bufs=1, space="DRAM") as dram,
        ):
            # ---------------- SBUF persistent tiles ----------------
            logits_sb = sb.tile([96, FREE], FP32)
            onehot_sb = sb.tile([96, FREE], FP32)
            ft_sb = sb.tile([4, BAND], BF16)
            rt_sb = sb.tile([4, STRIP], BF16)
            biasb_sb = sb.tile([128, BT], FP32)
            g_sb = sb.tile([96, 96], BF16)
            gs_sb = sb.tile([96, YL], BF16)
            ident_sb = sb.tile([32, 32], FP32)
            eyem20_sb = sb.tile([96, 96], FP32)
            info_sb = sb.tile([1, 2], mybir.dt.int32)
            kb_sb = sb.tile([128, BT * STRIP], BF16)       # 48KB/part resident
            zero_sb = sb.tile([128, FREE], BF16)

            nc.sync.dma_start(logits_sb[:], logits_d[:])
            nc.sync.dma_start(onehot_sb[:], onehot_d[:])
            nc.sync.dma_start(ft_sb[:], ft_d[:])
            nc.sync.dma_start(rt_sb[:], rt_d[:])
            nc.sync.dma_start(biasb_sb[:], biasb_d[:])
            nc.sync.dma_start(g_sb[:], g_d[:])
            nc.sync.dma_start(gs_sb[:], gs_d[:])
            nc.sync.dma_start(ident_sb[:], ident_d[:])
            nc.sync.dma_start(eyem20_sb[:], eyem20_d[:])
            nc.sync.dma_start(info_sb[:], info_d[:])
            nc.vector.memset(zero_sb[:], 0.0)

            band0 = nc.values_load(
                info_sb[0:1, 0:1], min_val=0, max_val=(NCORES - 1) * STRIP,
                skip_runtime_bounds_check=True,
            )

            # ---------------- DRAM scratch ----------------
            qout = dram.tile([STRIP, C], BF16)
            qpad = dram.tile([QPAD_ROWS, C], BF16)
            t_dram = dram.tile([YL, 96 * C], BF16)
            ce_in = dram.tile([1, 1], FP32)
            ce_out = dram.tile([1, 1], FP32)

            # zero the qpad margins (once; iterations only overwrite the middle)
            for r0 in (0, PAD + N):
                nc.sync.dma_start(
                    qpad[r0:r0 + PAD, :].rearrange("(t p) c -> p t c", p=128),
                    zero_sb[:, 0:(PAD // 128) * C].rearrange(
                        "p (t c) -> p t c", c=C),
                )

            # ---------------- Kb generation ----------------
            # rgb inner products (bf16 matmul) -> exp -> multiply by the
            # host-precomputed spatial table S (a constant, like G)
            with (
                tc.tile_pool(name="gen_ps", bufs=2, space="PSUM") as gen_ps,
                tc.tile_pool(name="gen_sb", bufs=3) as gen_sb,
            ):
                for t in range(BT if gen else 0):
                    ps_g = gen_ps.tile([128, STRIP], FP32, tag="gen")
                    for (o, w) in CHUNKS3:
                        nc.tensor.matmul(
                            ps_g[:, o:o + w],
                            ft_sb[0:4, t * 128:(t + 1) * 128],
                            rt_sb[0:4, o:o + w],
                            start=True, stop=True,
                        )
                    e_g = gen_sb.tile([128, STRIP], BF16, tag="eg")
                    nc.scalar.activation(
                        e_g[:], ps_g[:], AF.Exp, bias=biasb_sb[:, t:t + 1])
                    s_g = gen_sb.tile([128, STRIP], BF16, tag="sg")
                    nc.sync.dma_start(
                        s_g[:], s_d[:, t * STRIP:(t + 1) * STRIP])
                    nc.vector.tensor_mul(
                        kb_sb[:, t * STRIP:(t + 1) * STRIP], e_g[:], s_g[:])

            # ---------------- Q0 + CE ----------------
            e0 = sb.tile([96, FREE], FP32)
            s12 = sb.tile([96, YL], FP32)
            r12 = sb.tile([96, YL], FP32)
            lse = sb.tile([96, YL], FP32)
            dot = sb.tile([96, FREE], FP32)
            d12 = sb.tile([96, YL], FP32)
            ce96 = sb.tile([96, 1], FP32)
            ones96 = sb.tile([96, 1], FP32)
            ones1 = sb.tile([1, 96], FP32)
            ce_sb = sb.tile([1, 1], FP32)
            ce_all = sb.tile([1, 1], FP32)
            ce_bcast = sb.tile([96, 1], FP32)
            qA = sb.tile([96, FREE], FP32)
            qB = sb.tile([96, FREE], FP32)

            nc.vector.memset(ones96[:], 1.0)
            nc.vector.memset(ones1[:], 1.0)

            def bcast(t12):
                return t12[:].rearrange(
                    "p (y one) -> p y one", one=1).broadcast_to([96, YL, C])

            def as3(t):
                return t[:].rearrange("p (y c) -> p y c", c=C)

            nc.scalar.activation(e0[:], logits_sb[:], AF.Exp)
            nc.vector.tensor_reduce(
                s12[:], as3(e0), axis=AX.X, op=ALU.add)
            nc.vector.reciprocal(r12[:], s12[:])
            nc.vector.tensor_mul(as3(qA), as3(e0), bcast(r12))
            # ce partial
            nc.scalar.activation(lse[:], s12[:], AF.Ln)
            nc.vector.tensor_mul(dot[:], logits_sb[:], onehot_sb[:])
            nc.vector.tensor_reduce(
                d12[:], as3(dot), axis=AX.X, op=ALU.add)
            nc.vector.tensor_sub(d12[:], lse[:], d12[:])
            nc.vector.tensor_reduce(ce96[:], d12[:], axis=AX.X, op=ALU.add)
            with tc.tile_pool(name="ce_ps", bufs=1, space="PSUM") as ce_ps:
                cep = ce_ps.tile([1, 1], FP32)
                nc.tensor.matmul(cep[:], ce96[:], ones96[:], start=True, stop=True)
                nc.scalar.activation(ce_sb[:], cep[:], AF.Copy, scale=1.0 / N)
            nc.sync.dma_start(ce_in[:], ce_sb[:])
            if sim_single:
                nc.sync.dma_start(ce_out[:], ce_in[:])
            else:
                nc.gpsimd.collective_compute(
                    "AllReduce", ALU.add,
                    replica_groups=[list(range(NCORES))],
                    ins=[ce_in.opt()], outs=[ce_out.opt()],
                )
            nc.sync.dma_start(ce_all[:], ce_out[:])
            with tc.tile_pool(name="ceb_ps", bufs=1, space="PSUM") as ceb_ps:
                cebp = ceb_ps.tile([96, 1], FP32)
                nc.tensor.matmul(cebp[:], ones1[:], ce_all[:], start=True, stop=True)
                nc.vector.tensor_copy(ce_bcast[:], cebp[:])

            # ---------------- iteration tiles ----------------
            q16 = sb.tile([96, FREE], BF16)
            qb16 = sb.tile([128, BT * C], BF16)
            qg = sb.tile([96, 96 * C], BF16)
            tcp = sb.tile([YL, 96 * C], BF16)
            tp = sb.tile([96, FREE], BF16)
            msgT = sb.tile([21, STRIP], FP32)
            mg = sb.tile([96, FREE], FP32)
            msum = sb.tile([96, FREE], FP32)
            z1 = sb.tile([96, FREE], FP32)
            z2 = sb.tile([96, FREE], FP32)
            ez = sb.tile([96, FREE], FP32)
            negm = sb.tile([96, YL], FP32)

            q_cur, q_nxt = qA, qB

            with (
                tc.tile_pool(name="mm_ps", bufs=1, space="PSUM") as mm_ps,
                tc.tile_pool(name="kg_ps", bufs=1, space="PSUM") as kg_ps,
                tc.tile_pool(name="tr_ps", bufs=1, space="PSUM") as tr_ps,
            ):
                for it in range(n_iters):
                    # publish strip (bf16) -> global order [1152, 21]
                    nc.vector.tensor_copy(q16[:], q_cur[:])
                    nc.sync.dma_start(
                        qout[:].rearrange("(y x) c -> x y c", x=96),
                        q16[:].rearrange("p (y c) -> p y c", c=C),
                    )
                    if sim_single:
                        nc.sync.dma_start(qpad[PAD:PAD + STRIP, :], qout[:])
                    else:
                        # AllGather straight into the middle of qpad
                        nc.gpsimd.collective_compute(
                            "AllGather", ALU.bypass,
                            replica_groups=[list(range(NCORES))],
                            ins=[qout.opt()], outs=[qpad[PAD:PAD + N, :]],
                        )

                    # center tiles of the band are my own strip: read them
                    # from the local qout copy, no AllGather dependency
                    nc.sync.dma_start(
                        qb16[:, HT * C:(HT + TS) * C].rearrange(
                            "p (t c) -> p t c", c=C),
                        qout[:].rearrange("(t p) c -> p t c", p=128),
                    )
                    # halo tiles come out of the AllGathered padded buffer
                    nc.gpsimd.dma_start(
                        qb16[:, 0:HT * C].rearrange("p (t c) -> p t c", c=C),
                        qpad[bass.ds(band0, PAD), :].rearrange(
                            "(t p) c -> p t c", p=128),
                    )
                    nc.gpsimd.dma_start(
                        qb16[:, (HT + TS) * C:].rearrange(
                            "p (t c) -> p t c", c=C),
                        qpad[bass.ds(band0 + PAD + STRIP, PAD), :].rearrange(
                            "(t p) c -> p t c", p=128),
                    )

                    # ---- Kb matmul: psumT[c, s] += Q_band_t^T @ Kb_t
                    # center tiles first (no AG dependency), halo after;
                    # chunk-outer so each chunk's psum evacuates while the
                    # next chunk's matmuls run
                    psT = mm_ps.tile([21, STRIP], FP32, tag="mm")
                    for (o, w) in CHUNKS3:
                        order = CENTER + HALO
                        for n, t in enumerate(order):
                            nc.tensor.matmul(
                                psT[:, o:o + w],
                                qb16[:, t * C:(t + 1) * C],
                                kb_sb[:, t * STRIP + o: t * STRIP + o + w],
                                start=(n == 0), stop=(n == BT - 1),
                            )
                        nc.vector.tensor_copy(
                            msgT[:, o:o + w], psT[:, o:o + w])

                    # ---- Kg path: y-conv restricted to my strip rows via
                    # G_strip, then x-conv, with a DRAM bounce to repartition
                    nc.sync.dma_start(
                        qg[:].rearrange("p (x c) -> p x c", c=C),
                        qpad[PAD:PAD + N, :].rearrange("(y x) c -> y x c", x=96))
                    psA = kg_ps.tile([YL, 2048], FP32, tag="kg")
                    for (o, w) in CHUNKS4:
                        nc.tensor.matmul(
                            psA[:, o:o + w], gs_sb[:], qg[:, o:o + w],
                            start=True, stop=True,
                        )
                    nc.scalar.activation(tcp[:], psA[:, 0:96 * C], AF.Copy)
                    nc.sync.dma_start(t_dram[:], tcp[:])
                    nc.sync.dma_start(
                        tp[:].rearrange("p (y c) -> p y c", c=C),
                        t_dram[:].rearrange("y (x c) -> x y c", c=C))
                    # x-conv, then -20*q_cur accumulated into the same psum
                    # (Kb/Kg carry the COMPAT factor from the host tables)
                    psB = kg_ps.tile([96, FREE], FP32, tag="kg")
                    nc.tensor.matmul(
                        psB[:], g_sb[:], tp[:], start=True, stop=False)
                    nc.tensor.matmul(
                        psB[:], eyem20_sb[:], q_cur[:],
                        start=False, stop=True, skip_group_check=True,
                    )
                    nc.scalar.activation(mg[:], psB[:], AF.Copy)

                    # ---- transpose msgT -> strip domain [96, (y, c)]
                    pstr = tr_ps.tile([96, FREE], FP32, tag="tr")
                    for y in range(YL):
                        nc.tensor.transpose(
                            pstr[:, y * C:(y + 1) * C],
                            msgT[:, y * 96:(y + 1) * 96],
                            ident_sb[0:21, 0:21],
                        )

                    # ---- combine + softmax update
                    # z = logits + (10*msg_b)^T + 10*msg_g - 20*q
                    nc.vector.tensor_add(msum[:], pstr[:], mg[:])
                    nc.vector.tensor_add(z1[:], msum[:], logits_sb[:])
                    nc.vector.tensor_reduce(
                        negm[:], as3(z1), axis=AX.X, op=ALU.max, negate=True)
                    nc.vector.tensor_add(as3(z2), as3(z1), bcast(negm))
                    nc.scalar.activation(ez[:], z2[:], AF.Exp)
                    nc.vector.tensor_reduce(
                        s12[:], as3(ez), axis=AX.X, op=ALU.add)
                    nc.vector.reciprocal(r12[:], s12[:])
                    nc.vector.tensor_mul(as3(q_nxt), as3(ez), bcast(r12))
                    q_cur, q_nxt = q_nxt, q_cur

            # ---------------- output ----------------
            outs = sb.tile([96, FREE], FP32)
            nc.vector.tensor_scalar_add(outs[:], q_cur[:], ce_bcast[:])
            nc.sync.dma_start(out_d[:], outs[:])

    nc.compile()
    return nc


def host_prepare(logits, labels, image):
    """Build the 8 per-core input maps."""
    logits_nc = np.ascontiguousarray(
        np.asarray(logits, np.float32)[0].reshape(C, N).T)      # [N, C]
    labels_n = np.asarray(labels).reshape(N).astype(np.int64)
    rgb = np.asarray(image, np.float32)[0].transpose(1, 2, 0).reshape(N, 3)

    onehot = np.zeros((N, C), np.float32)
    onehot[np.arange(N), labels_n] = 1.0

    yy, xx = np.meshgrid(np.arange(H), np.arange(W), indexing="ij")
    pos = np.stack([yy, xx], -1).reshape(N, 2).astype(np.float32)
    cpos = (pos - pos.mean(0)) / 3.0                             # spatial/sigma
    frgb = (rgb / 255.0).astype(np.float32)
    sqr = (frgb * frgb).sum(1)

    BF = ml_dtypes.bfloat16
    a = np.arange(H, dtype=np.float32)
    # sqrt(COMPAT)-scaled so (G x G) carries the full COMPAT factor
    G = (np.sqrt(COMPAT) * np.exp(
        -0.5 * ((a[:, None] - a[None, :]) / 64.0) ** 2)).astype(BF)
    ident = np.eye(32, dtype=np.float32)
    eyem20 = (-2.0 * COMPAT * np.eye(96)).astype(np.float32)
    # y-conv restricted to each core's strip rows: G_strip = G[:, r*YL:(r+1)*YL]

    def to_strip_dom(arr_nc, r):
        # [N, C] global rows -> [96, (y, c)] strip-domain layout
        s = arr_nc[r * STRIP:(r + 1) * STRIP].reshape(YL, 96, C)
        return np.ascontiguousarray(s.transpose(1, 0, 2).reshape(96, FREE))

    in_maps = []
    for r in range(NCORES):
        j = np.arange(r * STRIP - PAD, r * STRIP - PAD + BAND)
        valid = (j >= 0) & (j < N)
        jc = np.clip(j, 0, N - 1)
        ft = np.zeros((4, BAND), np.float32)
        ft[0:3, valid] = frgb[jc[valid]].T
        ft[3, valid] = 1.0
        i_idx = np.arange(r * STRIP, (r + 1) * STRIP)
        rt = np.concatenate(
            [frgb[i_idx].T, (-0.5 * sqr[i_idx])[None, :]], 0)
        biasb = np.where(valid, -0.5 * sqr[jc], 0.0).astype(np.float32)
        biasb = np.ascontiguousarray(biasb.reshape(BT, 128).T)   # [128, BT]
        # spatial table S[j, i] = exp(-0.5 * ||cpos_j - cpos_i||^2), 0 for
        # out-of-range j (also kills the padded rows)
        dsp = ((cpos[jc][:, None, :] - cpos[i_idx][None, :, :]) ** 2).sum(-1)
        S = (COMPAT * np.exp(-0.5 * dsp)).astype(np.float32)  # COMPAT folded in
        S[~valid] = 0.0
        S = np.ascontiguousarray(
            S.reshape(BT, 128, STRIP).transpose(1, 0, 2).reshape(
                128, BT * STRIP)).astype(BF)
        info = np.array([[r * STRIP, r * FREE]], np.int32)
        in_maps.append({
            "logits_dev": to_strip_dom(logits_nc, r),
            "onehot_dev": to_strip_dom(onehot, r),
            "ft_dev": ft.astype(BF),
            "rt_dev": np.ascontiguousarray(rt).astype(BF),
            "biasb_dev": biasb,
            "s_dev": S,
            "g_dev": np.ascontiguousarray(G),
            "gs_dev": np.ascontiguousarray(G[:, r * YL:(r + 1) * YL]),
            "ident_dev": ident,
            "eyem20_dev": eyem20,
            "info_dev": info,
        })
    return in_maps


def assemble_output(results):
    # per-core [96, FREE] strip-domain -> [1, C, H, W]
    q = np.zeros((N, C), np.float32)
    for r in range(NCORES):
        s = results[r]["out_strip"].reshape(96, YL, C).transpose(1, 0, 2)
        q[r * STRIP:(r + 1) * STRIP] = s.reshape(STRIP, C)
    return np.ascontiguousarray(q.T.reshape(1, C, H, W))


def kernel(logits, labels, image, num_classes, _trace=False):
    global _compiled
    if _compiled is None:
        _compiled = build_nc()
    in_maps = host_prepare(logits, labels, image)
    res = run_bass_kernel_spmd(
        _compiled, in_maps, list(range(NCORES)), trace=_trace)
    out = assemble_output(res.results)
    if _trace:
        return out, res
    return out

